# revision 10
# baseline (speedup 1.0000x reference)
"""Trainium2 Bass kernel for the double-Mamba block (nn_DoubleConv).

Sharding: 8 cores = 4 batches x 2 sequence halves. Each core processes
LC = 2048 + OV columns of its batch element; OV columns are burn-in
(delta >= 0.54 => per-step state decay <= e^-0.54, so OV/2 columns per
mamba layer push the truncation error below fp32 noise).

Layout: features on partitions, time on the free axis.
 - causal depthwise conv folded into in_proj: 4 accumulating PE matmuls
   with shifted rhs, lhsT_j = (conv_w[:, j] * W_in_xc).T
 - dA_n = Exp(delta * A[:, n]) on ScalarE (per-partition scale AP)
 - h_n via VectorE tensor_tensor_scan (fp32 state)
 - B/C rows broadcast across partitions via DRAM round-trip DMAs
 - y = sum_n C_n*h_n via PE identity-matmul PSUM accumulation
 - LayerNorm over the feature(partition) axis via ones/128 matmuls
"""
import numpy as np
from contextlib import ExitStack

import concourse.bass as bass
import concourse.bacc as bacc
import concourse.mybir as mybir
import concourse.tile as tile
from concourse.bass_utils import run_bass_kernel_spmd

F32 = mybir.dt.float32
F16 = mybir.dt.float16
AF = mybir.ActivationFunctionType
OP = mybir.AluOpType

D_STATE = 16
D_CONV = 4
B, L, IN_C, OUT_C = 4, 4096, 64, 128
OV = 128                      # burn-in columns (covers both layers)
LC = 2048 + OV                # per-core columns
LPAD = LC + 3                 # conv left-pad
BLK = 512                     # PSUM block


def _blocks(n, bs=BLK):
    return [(s, min(s + bs, n)) for s in range(0, n, bs)]


class _DmaRR:
    """Round-robin DMA issue over the two HWDGE engines."""

    def __init__(self, nc):
        self.engines = [nc.sync, nc.scalar]
        self.i = 0

    def __call__(self, out, in_):
        e = self.engines[self.i % 2]
        self.i += 1
        return e.dma_start(out, in_)


def _layer_norm(nc, pools, dma, lnrows_dram, row_base,
                h_raw, g_col, b_col, out_tile, out_off=0):
    """LN over the partition axis of h_raw [128, LC] (f16, SBUF).
    Writes normalized f16 into out_tile[:, out_off:out_off+LC]."""
    sb, mmp, vec = pools['sb'], pools['mm'], pools['vec']
    ones_over = pools['ones128']     # [128, 1] f16 of 1/128
    h_sq = sb.tile([128, LC], F16, tag="lnsq")
    nc.scalar.activation(h_sq[:], h_raw[:], AF.Square)
    # row slots chosen so every 2-input DVE op has equal base partitions
    vA = vec.tile([128, LC], F32, tag="vA", name="vA")
    vB = vec.tile([128, LC], F32, tag="vB", name="vB")
    v16 = vec.tile([64, LC], F16, tag="v16", name="v16")
    msq, var, mu = vA[0:1, :], vA[32:33, :], vA[64:65, :]
    mu2, s_row = vB[0:1, :], vB[64:65, :]
    mus_row = vA[96:97, :]
    for (s, e) in _blocks(LC):
        p1 = mmp.tile([1, BLK], F32, tag="mm")
        nc.tensor.matmul(p1[:, :e - s], ones_over[:], h_raw[:, s:e],
                         start=True, stop=True)
        nc.scalar.activation(mu[:, s:e], p1[:, :e - s], AF.Copy)
        p2 = mmp.tile([1, BLK], F32, tag="mm")
        nc.tensor.matmul(p2[:, :e - s], ones_over[:], h_sq[:, s:e],
                         start=True, stop=True)
        nc.scalar.activation(msq[:, s:e], p2[:, :e - s], AF.Copy)
    # var = msq - mu^2 ; s = rsqrt(var + eps); mus = mu*s     [1, LC] rows
    nc.scalar.activation(mu2, mu, AF.Square)
    nc.vector.tensor_tensor(out=var, in0=msq, in1=mu2, op=OP.subtract)
    nc.scalar.activation(s_row, var, AF.Abs_reciprocal_sqrt,
                         bias=pools['eps1'][:1, :])
    nc.vector.tensor_tensor(out=mus_row, in0=mu, in1=s_row, op=OP.mult)
    s16 = v16[0:1, :]
    mus16 = v16[32:33, :]
    nc.vector.tensor_copy(s16, s_row)
    nc.vector.tensor_copy(mus16, mus_row)
    dma(lnrows_dram.ap()[row_base:row_base + 1, :], s16)
    dma(lnrows_dram.ap()[row_base + 1:row_base + 2, :], mus16)
    s_bc = sb.tile([128, LC], F16, tag="lnbc0")
    mus_bc = sb.tile([128, LC], F16, tag="lnbc1")
    dma(s_bc[:], lnrows_dram.ap()[row_base:row_base + 1, :]
        .broadcast_to((128, LC)))
    dma(mus_bc[:], lnrows_dram.ap()[row_base + 1:row_base + 2, :]
        .broadcast_to((128, LC)))
    # out = ((h*s_bc) - mus_bc)*g + b
    t1 = sb.tile([128, LC], F16, tag="lnt1")
    nc.vector.tensor_tensor(out=t1[:], in0=h_raw[:], in1=s_bc[:], op=OP.mult)
    t2 = sb.tile([128, LC], F16, tag="lnsq")
    nc.vector.tensor_tensor(out=t2[:], in0=t1[:], in1=mus_bc[:],
                            op=OP.subtract)
    nc.vector.tensor_scalar(out=out_tile[:, out_off:out_off + LC],
                            in0=t2[:], scalar1=g_col[:], scalar2=b_col[:],
                            op0=OP.mult, op1=OP.add)


def _mamba(nc, pools, dma, W, lay, xin, xin_off, di, dtr,
           brow_dram, crow_dram, out_raw):
    """One mamba layer. xin: [d_model, xin_off+LC] f16 SBUF (3 valid pad
    columns before xin_off). out_raw: list of [128, LC] f16 tiles (one per
    128-feature group) receiving gated y (pre-out_proj)."""
    sb, mmp, yp = pools['sb'], pools['mm'], pools['yacc']
    n_grp = di // 128
    xc2 = [sb.tile([128, LC], F16, tag=f"xc_{g}") for g in range(n_grp)]
    sres = [sb.tile([128, LC], F16, tag=f"sres_{g}") for g in range(n_grp)]
    for g in range(n_grp):
        for (s, e) in _blocks(LC):
            mm = mmp.tile([128, BLK], F32, tag="mm")
            for j in range(D_CONV):
                nc.tensor.matmul(
                    mm[:, :e - s], W[f'Mj{lay}_{j}_{g}'][:],
                    xin[:, xin_off - 3 + j + s: xin_off - 3 + j + e],
                    start=(j == 0), stop=(j == D_CONV - 1))
            nc.scalar.activation(xc2[g][:, s:e], mm[:, :e - s], AF.Silu,
                                 bias=W[f'convb{lay}_{g}'][:])
            mm2 = mmp.tile([128, BLK], F32, tag="mm")
            nc.tensor.matmul(mm2[:, :e - s], W[f'Wres{lay}_{g}'][:],
                             xin[:, xin_off + s: xin_off + e],
                             start=True, stop=True)
            nc.scalar.activation(sres[g][:, s:e], mm2[:, :e - s], AF.Silu)
    # x_proj -> x_dbl rows [dtr+32, LC]
    nxd = dtr + 32
    xdbl16 = sb.tile([nxd, LC], F16, tag="xdbl")
    dt32 = sb.tile([dtr, LC], F32, tag="dt32")
    for (s, e) in _blocks(LC):
        mm = mmp.tile([nxd, BLK], F32, tag="mm")
        for g in range(n_grp):
            nc.tensor.matmul(mm[:, :e - s], W[f'xpT{lay}_{g}'][:],
                             xc2[g][:, s:e],
                             start=(g == 0), stop=(g == n_grp - 1))
        nc.scalar.activation(xdbl16[:, s:e], mm[:, :e - s], AF.Copy)
        nc.scalar.activation(dt32[:, s:e], mm[:dtr, :e - s], AF.Copy)
    dma(brow_dram.ap()[:, :], xdbl16[dtr:dtr + 16, :])
    dma(crow_dram.ap()[:, :], xdbl16[dtr + 16:dtr + 32, :])
    for g in range(n_grp):
        # delta' = ln(sigmoid(-(pre + dt_b))) = -softplus(pre + dt_b)
        # (compensated by sign-flipped A and B-row weights, set on host)
        delta = sb.tile([128, LC], F32, tag="delta")
        sigout = sb.tile([128, LC], F32, tag="sigout")
        for (s, e) in _blocks(LC):
            mm = mmp.tile([128, BLK], F32, tag="mm")
            nc.tensor.matmul(mm[:, :e - s], W[f'dtwT{lay}_{g}'][:],
                             dt32[:, s:e], start=True, stop=True)
            nc.scalar.activation(sigout[:, s:e], mm[:, :e - s],
                                 AF.Sigmoid, bias=W[f'dtbn{lay}_{g}'][:],
                                 scale=-1.0)
        nc.scalar.activation(delta[:], sigout[:], AF.Ln)
        w16 = sb.tile([128, LC], F16, tag="w16")
        nc.vector.tensor_tensor(out=w16[:], in0=delta[:], in1=xc2[g][:],
                                op=OP.mult)
        ytiles = [yp.tile([128, BLK], F32, tag="yacc", name="yacc")
                  for _ in _blocks(LC)]
        for n in range(16):
            dA = sb.tile([128, LC], F32, tag="dA")
            nc.scalar.activation(dA[:], delta[:], AF.Exp,
                                 scale=W[f'A{lay}_{g}'][:, n:n + 1])
            b_bc = sb.tile([128, LC], F16, tag="bbc")
            dma(b_bc[:], brow_dram.ap()[n:n + 1, :].broadcast_to((128, LC)))
            dBu = sb.tile([128, LC], F16, tag="dbu")
            nc.vector.tensor_tensor(out=dBu[:], in0=w16[:], in1=b_bc[:],
                                    op=OP.mult)
            h = sb.tile([128, LC], F16, tag="h")
            nc.vector.tensor_tensor_scan(h[:], dA[:], dBu[:], 0.0,
                                         OP.mult, OP.add)
            c_bc = sb.tile([128, LC], F16, tag="cbc")
            dma(c_bc[:], crow_dram.ap()[n:n + 1, :].broadcast_to((128, LC)))
            q = sb.tile([128, LC], F16, tag="q")
            nc.vector.tensor_tensor(out=q[:], in0=h[:], in1=c_bc[:],
                                    op=OP.mult)
            for bi, (s, e) in enumerate(_blocks(LC)):
                nc.tensor.matmul(ytiles[bi][:, :e - s], pools['ident'][:],
                                 q[:, s:e], start=(n == 0), stop=(n == 15))
        # y + xc2*D, gate with silu(res)
        for bi, (s, e) in enumerate(_blocks(LC)):
            t1 = sb.tile([128, BLK], F32, tag="gt1")
            nc.vector.scalar_tensor_tensor(
                t1[:, :e - s], xc2[g][:, s:e], W[f'D{lay}_{g}'][:],
                ytiles[bi][:, :e - s], OP.mult, OP.add)
            nc.vector.tensor_tensor(out=out_raw[g][:, s:e],
                                    in0=t1[:, :e - s], in1=sres[g][:, s:e],
                                    op=OP.mult)


def build_nc():
    nc = bacc.Bacc("TRN2", target_bir_lowering=False, debug=False)
    dram_w = {}

    def reg(name, shape, dt):
        dram_w[name] = nc.dram_tensor(name, list(shape), dt,
                                      kind="ExternalInput")

    x_d = nc.dram_tensor("x_t", [IN_C, LPAD], F16, kind="ExternalInput")
    out_d = nc.dram_tensor("out", [128, 2048], F32, kind="ExternalOutput")
    for lay, (dm, di, dtr) in {1: (IN_C, 128, 4), 2: (OUT_C, 256, 8)}.items():
        for g in range(di // 128):
            for j in range(D_CONV):
                reg(f'Mj{lay}_{j}_{g}', [dm, 128], F16)
            reg(f'Wres{lay}_{g}', [dm, 128], F16)
            reg(f'convb{lay}_{g}', [128, 1], F32)
            reg(f'xpT{lay}_{g}', [128, dtr + 32], F16)
            reg(f'dtwT{lay}_{g}', [dtr, 128], F32)
            reg(f'dtbn{lay}_{g}', [128, 1], F32)
            reg(f'A{lay}_{g}', [128, 16], F32)
            reg(f'D{lay}_{g}', [128, 1], F32)
            reg(f'WoT{lay}_{g}', [128, 128], F16)
        reg(f'ln{lay}_g', [128, 1], F32)
        reg(f'ln{lay}_b', [128, 1], F32)
    reg('WlinT', [IN_C, 128], F16)
    reg('linb', [128, 1], F32)
    reg('lnr_g', [128, 1], F32)
    reg('lnr_b', [128, 1], F32)
    reg('ident', [128, 128], F16)
    reg('ones128', [128, 1], F16)
    reg('eps1', [128, 1], F32)

    brow1 = nc.dram_tensor("brow1", [16, LC], F16)
    crow1 = nc.dram_tensor("crow1", [16, LC], F16)
    brow2 = nc.dram_tensor("brow2", [16, LC], F16)
    crow2 = nc.dram_tensor("crow2", [16, LC], F16)
    lnrows = nc.dram_tensor("lnrows", [6, LC], F16)

    with tile.TileContext(nc) as tc, ExitStack() as ctx:
        sb1 = ctx.enter_context(tc.tile_pool(name="sb1", bufs=1))
        sb2 = ctx.enter_context(tc.tile_pool(name="sb2", bufs=2))
        mmp = ctx.enter_context(tc.tile_pool(name="mmp", bufs=2,
                                             space="PSUM"))
        yp = ctx.enter_context(tc.tile_pool(name="yp", bufs=5,
                                            space="PSUM"))
        vec = ctx.enter_context(tc.tile_pool(name="vec", bufs=1))
        wpool = ctx.enter_context(tc.tile_pool(name="wp", bufs=1))
        dma = _DmaRR(nc)

        Wt = {}
        for name, t in dram_w.items():
            tl = wpool.tile(list(t.shape), t.dtype, tag=name)
            dma(tl[:], t.ap())
            Wt[name] = tl
        class _SbMux:
            P1 = {"xc_0", "xc_1", "sres_0", "sres_1", "xdbl", "dt32",
                  "delta", "sigout", "w16", "r_n", "h1n", "h2n", "lnbc0", "lnbc1",
                  "lnsq", "lnt1"}

            def tile(self, shape, dt, tag):
                pool = sb1 if tag in self.P1 else sb2
                return pool.tile(shape, dt, tag=tag, name=tag)

        pools = dict(sb=_SbMux(), mm=mmp, yacc=yp, vec=vec,
                     ident=Wt['ident'], ones128=Wt['ones128'],
                     eps1=Wt['eps1'])

        xt = wpool.tile([IN_C, LPAD], F16, tag="xt")
        dma(xt[:], x_d.ap())

        # residual linear branch
        r_raw = pools["sb"].tile([128, LC], F16, tag="rawbuf")
        for (s, e) in _blocks(LC):
            mm = mmp.tile([128, BLK], F32, tag="mm")
            nc.tensor.matmul(mm[:, :e - s], Wt['WlinT'][:],
                             xt[:, 3 + s: 3 + e], start=True, stop=True)
            nc.scalar.activation(r_raw[:, s:e], mm[:, :e - s], AF.Identity,
                                 bias=Wt['linb'][:])
        r_n = pools["sb"].tile([128, LC], F16, tag="r_n")
        _layer_norm(nc, pools, dma, lnrows, 0, r_raw,
                    Wt['lnr_g'], Wt['lnr_b'], r_n)

        # mamba 1
        m1_raw = [pools["sb"].tile([128, LC], F16, tag="mraw")]
        _mamba(nc, pools, dma, Wt, 1, xt, 3, 128, 4, brow1, crow1, m1_raw)
        o1_raw = pools["sb"].tile([128, LC], F16, tag="rawbuf")
        for (s, e) in _blocks(LC):
            mm = mmp.tile([128, BLK], F32, tag="mm")
            nc.tensor.matmul(mm[:, :e - s], Wt['WoT1_0'][:],
                             m1_raw[0][:, s:e], start=True, stop=True)
            nc.scalar.activation(o1_raw[:, s:e], mm[:, :e - s], AF.Copy)
        h1n = pools["sb"].tile([128, LC + 3], F16, tag="h1n")
        nc.vector.memset(h1n[:, 0:3], 0.0)
        _layer_norm(nc, pools, dma, lnrows, 2, o1_raw,
                    Wt['ln1_g'], Wt['ln1_b'], h1n, out_off=3)

        # mamba 2
        m2_raw = [pools["sb"].tile([128, LC], F16, tag="mraw") for _ in range(2)]
        _mamba(nc, pools, dma, Wt, 2, h1n, 3, 256, 8, brow2, crow2, m2_raw)
        o2_raw = pools["sb"].tile([128, LC], F16, tag="rawbuf")
        for (s, e) in _blocks(LC):
            mm = mmp.tile([128, BLK], F32, tag="mm")
            for g in range(2):
                nc.tensor.matmul(mm[:, :e - s], Wt[f'WoT2_{g}'][:],
                                 m2_raw[g][:, s:e],
                                 start=(g == 0), stop=(g == 1))
            nc.scalar.activation(o2_raw[:, s:e], mm[:, :e - s], AF.Copy)
        h2n = pools["sb"].tile([128, LC], F16, tag="h2n")
        _layer_norm(nc, pools, dma, lnrows, 4, o2_raw,
                    Wt['ln2_g'], Wt['ln2_b'], h2n)

        # final: out = r + h2n (last 2048 columns)
        for (s, e) in _blocks(2048):
            fin = pools["sb"].tile([128, BLK], F32, tag="fin")
            nc.vector.tensor_tensor(out=fin[:, :e - s],
                                    in0=r_n[:, OV + s:OV + e],
                                    in1=h2n[:, OV + s:OV + e], op=OP.add)
            dma(out_d.ap()[:, s:e], fin[:, :e - s])

    nc.compile()
    return nc


_NC_CACHE = {}


def _get_nc():
    if 'nc' not in _NC_CACHE:
        _NC_CACHE['nc'] = build_nc()
    return _NC_CACHE['nc']


def _host_weights(params):
    out = {}
    for lay, key, di in ((1, 'm1', 128), (2, 'm2', 256)):
        P = params[key]
        Win = np.asarray(P['in_proj'])          # [2di, dm]
        convw = np.asarray(P['conv_w'])         # [di, 4]
        for g in range(di // 128):
            rows = slice(g * 128, (g + 1) * 128)
            for j in range(D_CONV):
                Mj = (convw[rows, j:j + 1] * Win[:di][rows]).T
                out[f'Mj{lay}_{j}_{g}'] = np.ascontiguousarray(Mj).astype(np.float16)
            out[f'Wres{lay}_{g}'] = np.ascontiguousarray(
                Win[di:][rows].T).astype(np.float16)
            out[f'convb{lay}_{g}'] = np.asarray(P['conv_b'])[rows, None].astype(np.float32)
            xp = np.asarray(P['x_proj']).copy()
            dtr = {1: 4, 2: 8}[lay]
            xp[dtr:dtr + 16] = -xp[dtr:dtr + 16]      # B rows sign-flipped
            out[f'xpT{lay}_{g}'] = np.ascontiguousarray(
                xp[:, rows].T).astype(np.float16)
            out[f'dtwT{lay}_{g}'] = np.ascontiguousarray(
                np.asarray(P['dt_w'])[rows].T).astype(np.float32)
            out[f'dtbn{lay}_{g}'] = (-np.asarray(P['dt_b']))[rows, None].astype(np.float32)
            out[f'A{lay}_{g}'] = np.ascontiguousarray(
                np.exp(np.asarray(P['A_log'])[rows])).astype(np.float32)
            out[f'D{lay}_{g}'] = np.asarray(P['D'])[rows, None].astype(np.float32)
            out[f'WoT{lay}_{g}'] = np.ascontiguousarray(
                np.asarray(P['out_proj'])[:, rows].T).astype(np.float16)
    out['ln1_g'] = np.asarray(params['ln1_g'])[:, None].astype(np.float32)
    out['ln1_b'] = np.asarray(params['ln1_b'])[:, None].astype(np.float32)
    out['ln2_g'] = np.asarray(params['ln2_g'])[:, None].astype(np.float32)
    out['ln2_b'] = np.asarray(params['ln2_b'])[:, None].astype(np.float32)
    out['WlinT'] = np.ascontiguousarray(
        np.asarray(params['lin_w']).T).astype(np.float16)
    out['linb'] = np.asarray(params['lin_b'])[:, None].astype(np.float32)
    out['lnr_g'] = np.asarray(params['ln_r_g'])[:, None].astype(np.float32)
    out['lnr_b'] = np.asarray(params['ln_r_b'])[:, None].astype(np.float32)
    out['ident'] = np.eye(128, dtype=np.float16)
    out['ones128'] = np.full((128, 1), 1.0 / 128, np.float16)
    out['eps1'] = np.full((128, 1), 1e-5, np.float32)
    return out


def kernel(x, params):
    x = np.asarray(x, np.float32)
    nc = _get_nc()
    wts = _host_weights(params)
    in_maps = []
    for c in range(8):
        b, half = c // 2, c % 2
        start = half * 2048 - OV
        xs = np.zeros((IN_C, LPAD), np.float16)
        lo = start - 3
        src_lo = max(lo, 0)
        src_hi = start + LC
        xs[:, src_lo - lo: src_lo - lo + (src_hi - src_lo)] = \
            x[b, src_lo:src_hi, :].T.astype(np.float16)
        m = dict(wts)
        m['x_t'] = xs
        in_maps.append(m)
    res = run_bass_kernel_spmd(nc, in_maps, core_ids=list(range(8)))
    _NC_CACHE['last_result'] = res
    out = np.zeros((B, L, 128), np.float32)
    for c in range(8):
        b, half = c // 2, c % 2
        out[b, half * 2048:(half + 1) * 2048, :] = res.results[c]['out'].T
    return out
